# revision 12
# baseline (speedup 1.0000x reference)
"""Trainium2 Bass kernel for the EnsembleGRU problem (8-core SPMD).

Math (per ensemble e, flattened batch n, timestep w):
    y  = x @ weight_linear.T + bias_linear          (P=72 proj)
    gx = y @ w_ih.T + b_ih                          (3 gates)
which composes to gx = x @ W_eff.T + b_eff with
    W_eff[e,g,f] = sum_p w_ih[e,g,p] * weight_linear[e,p,f]
    b_eff[e,g]   = sum_p w_ih[e,g,p] * bias_linear[e,p] + b_ih[e,g]
then the GRU (hidden_size=1) scan:
    r = sigmoid(gx0 + w0*h + bh0);  z = sigmoid(gx1 + w1*h + bh1)
    n = tanh(gx2 + r*(w2*h + bh2));  h' = (1-z)*n + z*h

Device plan per core (2 ensembles):
  - HWDGE-load x[w] f32 -> SBUF [128 n, (e,c,f)]
  - engine cast f32->f16 with col reorder -> [128 n, (c,e,f)]
  - xbar DMA-transpose per 128-col chunk -> xT [128 (e,f), 128 n] f16
  - PE matmuls with per-e weight stacks -> PSUM gx [128 chains, (q,c,g)] f32
  - recurrence on DVE (affine_then_add / affine_mul_reduce) + ACT (sigmoid/tanh)
  - h' [128, 8] streamed out per step

Chain layout (p = partition, q = P/Q half, c = 128-chunk of n):
  p<64:  e=0, n = 128c + (p%64) + 64*q
  p>=64: e=1, n = 128c + (p%64) + 64*(1-q)
"""
import numpy as np
from contextlib import ExitStack

W_STEPS, E, B, I, F = 128, 16, 64, 8, 64
N = B * I            # 512
E_LOC = 2            # ensembles per core
N_CORES = 8
NCHUNK = 4           # n chunks of 128
PSUM_SLOTS = 8       # gx pipeline depth (one slot per PSUM bank)


def _chain_maps():
    """e_idx, n_idx arrays [128, 2, 4] for (p, q, c) -> (e_loc, n)."""
    p = np.arange(128)
    e = (p // 64).astype(np.int64)
    pl = p % 64
    e_idx = np.zeros((128, 2, NCHUNK), np.int64)
    n_idx = np.zeros((128, 2, NCHUNK), np.int64)
    for q in range(2):
        for c in range(NCHUNK):
            half = np.where(e == 0, q, 1 - q)  # which 64-half of the chunk
            e_idx[:, q, c] = e
            n_idx[:, q, c] = 128 * c + pl + 64 * half
    return e_idx, n_idx


_E_IDX, _N_IDX = _chain_maps()


def _build_program(n_steps=W_STEPS, loop=1, mode="full"):
    import concourse.bass as bass
    import concourse.tile as tile
    from concourse import bacc, mybir

    nc = bacc.Bacc("TRN2", num_devices=N_CORES)
    f32, f16 = mybir.dt.float32, mybir.dt.float16
    AF = mybir.ActivationFunctionType

    # ---- DRAM I/O ----
    xin = nc.dram_tensor("xin", [n_steps, E_LOC, NCHUNK, 128, F], f32, kind="ExternalInput").ap()
    ident = nc.dram_tensor("ident", [128, 128], f16, kind="ExternalInput").ap()
    we16 = nc.dram_tensor("we16", [128, 6], f16, kind="ExternalInput").ap()
    scb = nc.dram_tensor("scb", [128, 8], f32, kind="ExternalInput").ap()  # w0,w1,w2,b0,b1,b2,bn,pad
    h0in = nc.dram_tensor("h0in", [128, 2 * NCHUNK], f32, kind="ExternalInput").ap()
    hout = nc.dram_tensor("hout", [n_steps, 128, 2 * NCHUNK], f32, kind="ExternalOutput").ap()

    QC = 2 * NCHUNK  # 8 free-dim chain columns

    with tile.TileContext(nc) as tc, ExitStack() as ctx:
        cpool = ctx.enter_context(tc.tile_pool(name="consts", bufs=1))
        x32p = ctx.enter_context(tc.tile_pool(name="x32", bufs=6))
        x16p = ctx.enter_context(tc.tile_pool(name="x16", bufs=6))
        xtp = ctx.enter_context(tc.tile_pool(name="xt", bufs=4))
        rzp = ctx.enter_context(tc.tile_pool(name="rz", bufs=3))
        smp = ctx.enter_context(tc.tile_pool(name="sm", bufs=3))
        hp = ctx.enter_context(tc.tile_pool(name="h", bufs=3))

        # constants
        idt = cpool.tile([128, 128], f16, name="idt")
        nc.sync.dma_start(idt[:], ident[:])
        we = cpool.tile([128, 6], f16, name="we")
        nc.sync.dma_start(we[:], we16[:])
        sc = cpool.tile([128, 8], f32, name="sc")
        nc.sync.dma_start(sc[:], scb[:])
        w0v, w1v, w2v = sc[:, 0:1], sc[:, 1:2], sc[:, 2:3]
        b0v, b1v, b2v, bnv = sc[:, 3:4], sc[:, 4:5], sc[:, 5:6], sc[:, 6:7]

        h_prev = cpool.tile([128, QC], f32, name="h_prev")
        nc.sync.dma_start(h_prev[:], h0in[:])

        # PSUM slots: one [128, 24] tensor pinned per bank (PE-write vs DVE-read
        # same-bank pairs are serialized by Tile only within a tensor)
        ps_banks = [nc.place_psum_tensor(f"gx{b}", [128, 24], f32, bank=b) for b in range(PSUM_SLOTS)]

        def ps_slot(w):
            return ps_banks[w % PSUM_SLOTS].ap()

        def g_ap(ps, g):
            # ps: [128, 24] = (g3, q2, c4): gate g's 8 chain-cols are contiguous
            return ps[:, 8 * g:8 * g + 8]

        for wg in range(loop * n_steps):
            w = wg % n_steps
            if mode == "rec":
                ps = ps_slot(wg)
                a_rz = rzp.tile([128, 2 * QC], f32, name="a_rz")
                nc.vector.affine_then_add(a_rz[:, 0:QC], h_prev[:], g_ap(ps, 0), w0v, b0v)
                nc.vector.affine_then_add(a_rz[:, QC:2 * QC], h_prev[:], g_ap(ps, 1), w1v, b1v)
                rz = rzp.tile([128, 2 * QC], f32, name="rz", tag="rzs")
                nc.scalar.activation(rz[:], a_rz[:], AF.Sigmoid)
                v = smp.tile([128, QC], f32, name="v")
                acc1 = smp.tile([128, 1], f32, name="acc1")
                nc.vector.affine_mul_reduce(v[:], acc1[:], h_prev[:], rz[:, 0:QC], w2v, b2v)
                t = smp.tile([128, QC], f32, name="t")
                nc.vector.affine_then_add(t[:], v[:], g_ap(ps, 2), 1.0, bnv)
                n_t = smp.tile([128, QC], f32, name="n_t")
                nc.scalar.activation(n_t[:], t[:], AF.Tanh)
                d = smp.tile([128, QC], f32, name="d")
                nc.vector.affine_then_add(d[:], n_t[:], h_prev[:], -1.0, 0.0)
                m = smp.tile([128, QC], f32, name="m")
                acc2 = smp.tile([128, 1], f32, name="acc2")
                nc.vector.affine_mul_reduce(m[:], acc2[:], d[:], rz[:, QC:2 * QC], 1.0, 0.0)
                h_new = hp.tile([128, QC], f32, name="h_new")
                nc.vector.affine_then_add(h_new[:], n_t[:], m[:], 1.0, 0.0)
                nc.scalar.dma_start(hout[w], h_new[:])
                h_prev = h_new
                continue
            # --- load x[w] as [128 n, (e, c, f)] f32 (2 DMAs, one per e) ---
            x32 = x32p.tile([128, E_LOC * NCHUNK * F], f32, name="x32")
            x32v = x32[:].rearrange("p (e c f) -> p e c f", e=E_LOC, c=NCHUNK, f=F)
            src = xin[w].rearrange("e c p f -> p e c f")
            nc.sync.dma_start(x32v, src)

            # --- cast f32 -> f16 with (e,c,f) -> (c,e,f) reorder ---
            x16 = x16p.tile([128, E_LOC * NCHUNK * F], f16, name="x16")
            x16v = x16[:].rearrange("p (c e f) -> p c e f", c=NCHUNK, e=E_LOC, f=F)
            x32r = x32[:].rearrange("p (e c f) -> p c e f", e=E_LOC, c=NCHUNK, f=F)
            cast_eng = [nc.vector, nc.scalar, nc.gpsimd, nc.gpsimd][wg % 4]
            if cast_eng is nc.scalar:
                nc.scalar.copy(x16v, x32r)
            else:
                cast_eng.tensor_copy(x16v, x32r)

            # --- xbar transposes [128 n, 128 (e,f)] -> [128 (e,f), 128 n]; all DMA on
            # one HWDGE queue (transpose/copy overlap across queues is a HW hazard) ---
            xt_sb = xtp.tile([128, 512], f16, name="xt_sb")
            for c in range(NCHUNK):
                nc.sync.dma_start(xt_sb[:, 128 * c:128 * (c + 1)], x16[:, 128 * c:128 * (c + 1)], transpose=True)

            # --- gates matmuls into PSUM slot ---
            ps = ps_slot(wg)
            ps3 = ps.rearrange("p (g qc) -> p qc g", g=3, qc=8)  # col = 8g + 4q + c
            for c in range(NCHUNK):
                lo, hi = xt_sb[:, 128 * c:128 * c + 64], xt_sb[:, 128 * c + 64:128 * c + 128]
                nc.tensor.matmul(ps3[0:64, c, :], lo, we[:, 0:3])        # e0, q0
                nc.tensor.matmul(ps3[64:128, 4 + c, :], lo, we[:, 3:6])  # e1, q1
                nc.tensor.matmul(ps3[0:64, 4 + c, :], hi, we[:, 0:3])    # e0, q1
                nc.tensor.matmul(ps3[64:128, c, :], hi, we[:, 3:6])      # e1, q0

            # --- recurrence step ---
            if mode == "bulk":
                a_r1 = rzp.tile([128, QC], f32, name="a_r1")
                nc.vector.affine_then_add(a_r1[:], h_prev[:], g_ap(ps, 0), w0v, b0v)
                nc.scalar.dma_start(hout[w], a_r1[:])
                continue
            # gate-z inputs are sign-flipped on host, so sigmoid gives zc = 1-z
            a_r = rzp.tile([128, QC], f32, name="a_r")
            nc.vector.affine_then_add(a_r[:], h_prev[:], g_ap(ps, 0), w0v, b0v)
            r_t = rzp.tile([128, QC], f32, name="r_t", tag="rts")
            nc.scalar.activation(r_t[:], a_r[:], AF.Sigmoid)
            a_z = rzp.tile([128, QC], f32, name="a_z", tag="azs")
            nc.vector.affine_then_add(a_z[:], h_prev[:], g_ap(ps, 1), w1v, b1v)
            zc = rzp.tile([128, QC], f32, name="zc", tag="zcs")
            nc.scalar.activation(zc[:], a_z[:], AF.Sigmoid)
            # B = z*h = h - zc*h   (GPSIMD, off the critical cycle)
            u_g = smp.tile([128, QC], f32, name="u_g")
            nc.gpsimd.tensor_mul(u_g[:], zc[:], h_prev[:])
            b_g = smp.tile([128, QC], f32, name="b_g")
            nc.gpsimd.tensor_sub(b_g[:], h_prev[:], u_g[:])

            v = smp.tile([128, QC], f32, name="v")
            acc1 = smp.tile([128, 1], f32, name="acc1")
            nc.vector.affine_mul_reduce(v[:], acc1[:], h_prev[:], r_t[:], w2v, b2v)
            t = smp.tile([128, QC], f32, name="t")
            nc.vector.affine_then_add(t[:], v[:], g_ap(ps, 2), 1.0, bnv)
            n_t = smp.tile([128, QC], f32, name="n_t")
            nc.scalar.activation(n_t[:], t[:], AF.Tanh)

            a_m = smp.tile([128, QC], f32, name="a_m")
            acc2 = smp.tile([128, 1], f32, name="acc2")
            nc.vector.affine_mul_reduce(a_m[:], acc2[:], n_t[:], zc[:], 1.0, 0.0)
            h_new = hp.tile([128, QC], f32, name="h_new")
            nc.vector.affine_then_add(h_new[:], a_m[:], b_g[:], 1.0, 0.0)

            nc.sync.dma_start(hout[w], h_new[:])
            h_prev = h_new

    nc.compile()
    return nc


_PROGRAM_CACHE = {}


def _get_program(n_steps=W_STEPS, loop=1, mode="full"):
    key = (n_steps, loop, mode)
    if key not in _PROGRAM_CACHE:
        _PROGRAM_CACHE[key] = _build_program(n_steps, loop, mode)
    return _PROGRAM_CACHE[key]


def _host_prep(inputs, state, weight_linear, bias_linear, w_ih, w_hh, b_ih, b_hh):
    """Per-core input maps."""
    n_steps = inputs.shape[0]
    W_eff = np.einsum("egp,epf->egf", w_ih.astype(np.float64), weight_linear.astype(np.float64))
    b_eff = np.einsum("egp,ep->eg", w_ih.astype(np.float64), bias_linear.astype(np.float64)) + b_ih
    W_eff = W_eff.astype(np.float32)
    b_eff = b_eff.astype(np.float32)

    x = inputs.reshape(n_steps, E, N, F)
    h_state = state[-1].reshape(E, N).astype(np.float32)

    in_maps = []
    for k in range(N_CORES):
        es = [2 * k, 2 * k + 1]
        # x slice -> [W, e, c, p, f]
        xs = x[:, es].reshape(n_steps, E_LOC, NCHUNK, 128, F).astype(np.float32)
        xs = np.ascontiguousarray(xs)

        # weight stacks [128 (e,f), 6] f16
        we = np.zeros((128, 6), np.float16)
        wsign = np.array([1.0, -1.0, 1.0], np.float32)  # z-gate negated -> sigmoid gives zc
        we[0:64, 0:3] = (W_eff[es[0]] * wsign[:, None]).T.astype(np.float16)   # [f, g]
        we[64:128, 3:6] = (W_eff[es[1]] * wsign[:, None]).T.astype(np.float16)

        # per-partition scale/bias vectors [128, 8]
        erow = np.repeat(np.array(es), 64)  # 128 rows -> global e
        scb = np.zeros((128, 8), np.float32)
        scb[:, 0] = w_hh[erow, 0]
        scb[:, 1] = -w_hh[erow, 1]
        scb[:, 2] = w_hh[erow, 2]
        scb[:, 3] = b_eff[erow, 0] + b_hh[erow, 0]
        scb[:, 4] = -(b_eff[erow, 1] + b_hh[erow, 1])
        scb[:, 5] = b_hh[erow, 2]
        scb[:, 6] = b_eff[erow, 2]

        # h0 in chain layout [128, (q, c)]
        h0 = h_state[2 * k + _E_IDX, _N_IDX].reshape(128, 2 * NCHUNK).astype(np.float32)

        in_maps.append({"xin": xs, "we16": we, "scb": scb, "h0in": h0,
                        "ident": np.eye(128, dtype=np.float16)})
    return in_maps


def _unpack_outputs(results):
    """results: list of dicts with 'hout' [W, 128, 8] -> full (W, E, B, I, 1)."""
    out = np.zeros((W_STEPS, E, N), np.float32)
    for k in range(N_CORES):
        h = results[k]["hout"].reshape(W_STEPS, 128, 2, NCHUNK)
        out[:, 2 * k + _E_IDX, _N_IDX] = h
    return out.reshape(W_STEPS, E, B, I, 1)


def kernel(inputs, state, weight_linear, bias_linear, w_ih, w_hh, b_ih, b_hh):
    from concourse.bass_utils import run_bass_kernel_spmd

    nc = _get_program()
    in_maps = _host_prep(np.asarray(inputs, np.float32), np.asarray(state, np.float32),
                         np.asarray(weight_linear, np.float32), np.asarray(bias_linear, np.float32),
                         np.asarray(w_ih, np.float32), np.asarray(w_hh, np.float32),
                         np.asarray(b_ih, np.float32), np.asarray(b_hh, np.float32))
    res = run_bass_kernel_spmd(nc, in_maps, core_ids=list(range(N_CORES)))
    return _unpack_outputs(res.results)


# revision 13
# speedup vs baseline: 1.0871x; 1.0871x over previous
"""Trainium2 Bass kernel for the EnsembleGRU problem (8-core SPMD).

Math (per ensemble e, flattened batch n, timestep w):
    y  = x @ weight_linear.T + bias_linear          (P=72 proj)
    gx = y @ w_ih.T + b_ih                          (3 gates)
which composes to gx = x @ W_eff.T + b_eff with
    W_eff[e,g,f] = sum_p w_ih[e,g,p] * weight_linear[e,p,f]
    b_eff[e,g]   = sum_p w_ih[e,g,p] * bias_linear[e,p] + b_ih[e,g]
then the GRU (hidden_size=1) scan:
    r = sigmoid(gx0 + w0*h + bh0);  z = sigmoid(gx1 + w1*h + bh1)
    n = tanh(gx2 + r*(w2*h + bh2));  h' = (1-z)*n + z*h

Device plan per core (2 ensembles):
  - HWDGE-load x[w] f32 -> SBUF [128 n, (e,c,f)]
  - engine cast f32->f16 with col reorder -> [128 n, (c,e,f)]
  - xbar DMA-transpose per 128-col chunk -> xT [128 (e,f), 128 n] f16
  - PE matmuls with per-e weight stacks -> PSUM gx [128 chains, (q,c,g)] f32
  - recurrence on DVE (affine_then_add / affine_mul_reduce) + ACT (sigmoid/tanh)
  - h' [128, 8] streamed out per step

Chain layout (p = partition, q = P/Q half, c = 128-chunk of n):
  p<64:  e=0, n = 128c + (p%64) + 64*q
  p>=64: e=1, n = 128c + (p%64) + 64*(1-q)
"""
import numpy as np
from contextlib import ExitStack

W_STEPS, E, B, I, F = 128, 16, 64, 8, 64
N = B * I            # 512
E_LOC = 2            # ensembles per core
N_CORES = 8
NCHUNK = 4           # n chunks of 128
PSUM_SLOTS = 8       # gx pipeline depth (one slot per PSUM bank)


def _chain_maps():
    """e_idx, n_idx arrays [128, 2, 4] for (p, q, c) -> (e_loc, n)."""
    p = np.arange(128)
    e = (p // 64).astype(np.int64)
    pl = p % 64
    e_idx = np.zeros((128, 2, NCHUNK), np.int64)
    n_idx = np.zeros((128, 2, NCHUNK), np.int64)
    for q in range(2):
        for c in range(NCHUNK):
            half = np.where(e == 0, q, 1 - q)  # which 64-half of the chunk
            e_idx[:, q, c] = e
            n_idx[:, q, c] = 128 * c + pl + 64 * half
    return e_idx, n_idx


_E_IDX, _N_IDX = _chain_maps()


def _build_program(n_steps=W_STEPS, loop=1, mode="full"):
    import concourse.bass as bass
    import concourse.tile as tile
    from concourse import bacc, mybir

    nc = bacc.Bacc("TRN2", num_devices=N_CORES)
    f32, f16 = mybir.dt.float32, mybir.dt.float16
    AF = mybir.ActivationFunctionType

    # ---- DRAM I/O ----
    xin = nc.dram_tensor("xin", [n_steps, E_LOC, NCHUNK, 128, F], f32, kind="ExternalInput").ap()
    ident = nc.dram_tensor("ident", [128, 128], f16, kind="ExternalInput").ap()
    we16 = nc.dram_tensor("we16", [128, 6], f16, kind="ExternalInput").ap()
    scb = nc.dram_tensor("scb", [128, 8], f32, kind="ExternalInput").ap()  # w0,w1,w2,b0,b1,b2,bn,pad
    h0in = nc.dram_tensor("h0in", [128, 2 * NCHUNK], f32, kind="ExternalInput").ap()
    hout = nc.dram_tensor("hout", [n_steps, 128, 2 * NCHUNK], f32, kind="ExternalOutput").ap()

    QC = 2 * NCHUNK  # 8 free-dim chain columns

    with tile.TileContext(nc) as tc, ExitStack() as ctx:
        cpool = ctx.enter_context(tc.tile_pool(name="consts", bufs=1))
        x32p = ctx.enter_context(tc.tile_pool(name="x32", bufs=6))
        x16p = ctx.enter_context(tc.tile_pool(name="x16", bufs=6))
        xtp = ctx.enter_context(tc.tile_pool(name="xt", bufs=4))
        rzp = ctx.enter_context(tc.tile_pool(name="rz", bufs=3))
        smp = ctx.enter_context(tc.tile_pool(name="sm", bufs=3))
        hp = ctx.enter_context(tc.tile_pool(name="h", bufs=3))

        # constants
        idt = cpool.tile([128, 128], f16, name="idt")
        nc.sync.dma_start(idt[:], ident[:])
        we = cpool.tile([128, 6], f16, name="we")
        nc.sync.dma_start(we[:], we16[:])
        sc = cpool.tile([128, 8], f32, name="sc")
        nc.sync.dma_start(sc[:], scb[:])
        w0v, w1v, w2v = sc[:, 0:1], sc[:, 1:2], sc[:, 2:3]
        b0v, b1v, b2v, bnv = sc[:, 3:4], sc[:, 4:5], sc[:, 5:6], sc[:, 6:7]

        h_prev = cpool.tile([128, QC], f32, name="h_prev")
        nc.sync.dma_start(h_prev[:], h0in[:])

        # PSUM slots: one [128, 24] tensor pinned per bank (PE-write vs DVE-read
        # same-bank pairs are serialized by Tile only within a tensor)
        ps_banks = [nc.place_psum_tensor(f"gx{b}", [128, 24], f32, bank=b) for b in range(PSUM_SLOTS)]

        def ps_slot(w):
            return ps_banks[w % PSUM_SLOTS].ap()

        def g_ap(ps, g):
            # ps: [128, 24] = (g3, q2, c4): gate g's 8 chain-cols are contiguous
            return ps[:, 8 * g:8 * g + 8]

        for wg in range(loop * n_steps):
            w = wg % n_steps
            if mode == "rec":
                ps = ps_slot(wg)
                a_rz = rzp.tile([128, 2 * QC], f32, name="a_rz")
                nc.vector.affine_then_add(a_rz[:, 0:QC], h_prev[:], g_ap(ps, 0), w0v, b0v)
                nc.vector.affine_then_add(a_rz[:, QC:2 * QC], h_prev[:], g_ap(ps, 1), w1v, b1v)
                rz = rzp.tile([128, 2 * QC], f32, name="rz", tag="rzs")
                nc.scalar.activation(rz[:], a_rz[:], AF.Sigmoid)
                v = smp.tile([128, QC], f32, name="v")
                acc1 = smp.tile([128, 1], f32, name="acc1")
                nc.vector.affine_mul_reduce(v[:], acc1[:], h_prev[:], rz[:, 0:QC], w2v, b2v)
                t = smp.tile([128, QC], f32, name="t")
                nc.vector.affine_then_add(t[:], v[:], g_ap(ps, 2), 1.0, bnv)
                n_t = smp.tile([128, QC], f32, name="n_t")
                nc.scalar.activation(n_t[:], t[:], AF.Tanh)
                d = smp.tile([128, QC], f32, name="d")
                nc.vector.affine_then_add(d[:], n_t[:], h_prev[:], -1.0, 0.0)
                m = smp.tile([128, QC], f32, name="m")
                acc2 = smp.tile([128, 1], f32, name="acc2")
                nc.vector.affine_mul_reduce(m[:], acc2[:], d[:], rz[:, QC:2 * QC], 1.0, 0.0)
                h_new = hp.tile([128, QC], f32, name="h_new")
                nc.vector.affine_then_add(h_new[:], n_t[:], m[:], 1.0, 0.0)
                nc.scalar.dma_start(hout[w], h_new[:])
                h_prev = h_new
                continue
            # --- load x[w] as [128 n, (e, c, f)] f32 (2 DMAs, one per e) ---
            x32 = x32p.tile([128, E_LOC * NCHUNK * F], f32, name="x32")
            x32v = x32[:].rearrange("p (e c f) -> p e c f", e=E_LOC, c=NCHUNK, f=F)
            src = xin[w].rearrange("e c p f -> p e c f")
            nc.sync.dma_start(x32v, src)

            # --- cast f32 -> f16 with (e,c,f) -> (c,e,f) reorder ---
            x16 = x16p.tile([128, E_LOC * NCHUNK * F], f16, name="x16")
            x16v = x16[:].rearrange("p (c e f) -> p c e f", c=NCHUNK, e=E_LOC, f=F)
            x32r = x32[:].rearrange("p (e c f) -> p c e f", e=E_LOC, c=NCHUNK, f=F)
            cast_eng = [nc.gpsimd, nc.gpsimd, nc.scalar, nc.vector][wg % 4]
            if cast_eng is nc.scalar:
                nc.scalar.copy(x16v, x32r)
            else:
                cast_eng.tensor_copy(x16v, x32r)

            # --- xbar transposes [128 n, 128 (e,f)] -> [128 (e,f), 128 n]; all DMA on
            # one HWDGE queue (transpose/copy overlap across queues is a HW hazard) ---
            xt_sb = xtp.tile([128, 512], f16, name="xt_sb")
            for c in range(NCHUNK):
                nc.sync.dma_start(xt_sb[:, 128 * c:128 * (c + 1)], x16[:, 128 * c:128 * (c + 1)], transpose=True)

            # --- gates matmuls into PSUM slot ---
            ps = ps_slot(wg)
            ps3 = ps.rearrange("p (g qc) -> p qc g", g=3, qc=8)  # col = 8g + 4q + c
            for c in range(NCHUNK):
                lo, hi = xt_sb[:, 128 * c:128 * c + 64], xt_sb[:, 128 * c + 64:128 * c + 128]
                nc.tensor.matmul(ps3[0:64, c, :], lo, we[:, 0:3])        # e0, q0
                nc.tensor.matmul(ps3[64:128, 4 + c, :], lo, we[:, 3:6])  # e1, q1
                nc.tensor.matmul(ps3[0:64, 4 + c, :], hi, we[:, 0:3])    # e0, q1
                nc.tensor.matmul(ps3[64:128, c, :], hi, we[:, 3:6])      # e1, q0

            # --- recurrence step ---
            if mode == "bulk":
                a_r1 = rzp.tile([128, QC], f32, name="a_r1")
                nc.vector.affine_then_add(a_r1[:], h_prev[:], g_ap(ps, 0), w0v, b0v)
                nc.scalar.dma_start(hout[w], a_r1[:])
                continue
            # gate-z inputs are sign-flipped on host, so sigmoid gives zc = 1-z
            a_r = rzp.tile([128, QC], f32, name="a_r")
            nc.vector.affine_then_add(a_r[:], h_prev[:], g_ap(ps, 0), w0v, b0v)
            r_t = rzp.tile([128, QC], f32, name="r_t", tag="rts")
            nc.scalar.activation(r_t[:], a_r[:], AF.Sigmoid)
            a_z = rzp.tile([128, QC], f32, name="a_z", tag="azs")
            nc.vector.affine_then_add(a_z[:], h_prev[:], g_ap(ps, 1), w1v, b1v)
            zc = rzp.tile([128, QC], f32, name="zc", tag="zcs")
            nc.scalar.activation(zc[:], a_z[:], AF.Sigmoid)
            v = smp.tile([128, QC], f32, name="v")
            acc1 = smp.tile([128, 1], f32, name="acc1")
            nc.vector.affine_mul_reduce(v[:], acc1[:], h_prev[:], r_t[:], w2v, b2v)
            t = smp.tile([128, QC], f32, name="t")
            nc.vector.affine_then_add(t[:], v[:], g_ap(ps, 2), 1.0, bnv)
            n_t = smp.tile([128, QC], f32, name="n_t")
            nc.scalar.activation(n_t[:], t[:], AF.Tanh)

            # h' = h - zc*(h - n)
            d = smp.tile([128, QC], f32, name="d")
            nc.vector.affine_then_add(d[:], n_t[:], h_prev[:], -1.0, 0.0)
            q = smp.tile([128, QC], f32, name="q")
            acc2 = smp.tile([128, 1], f32, name="acc2")
            nc.vector.affine_mul_reduce(q[:], acc2[:], d[:], zc[:], 1.0, 0.0)
            h_new = hp.tile([128, QC], f32, name="h_new")
            nc.vector.affine_then_add(h_new[:], q[:], h_prev[:], -1.0, 0.0)

            nc.sync.dma_start(hout[w], h_new[:])
            h_prev = h_new

    nc.compile()
    return nc


_PROGRAM_CACHE = {}


def _get_program(n_steps=W_STEPS, loop=1, mode="full"):
    key = (n_steps, loop, mode)
    if key not in _PROGRAM_CACHE:
        _PROGRAM_CACHE[key] = _build_program(n_steps, loop, mode)
    return _PROGRAM_CACHE[key]


def _host_prep(inputs, state, weight_linear, bias_linear, w_ih, w_hh, b_ih, b_hh):
    """Per-core input maps."""
    n_steps = inputs.shape[0]
    W_eff = np.einsum("egp,epf->egf", w_ih.astype(np.float64), weight_linear.astype(np.float64))
    b_eff = np.einsum("egp,ep->eg", w_ih.astype(np.float64), bias_linear.astype(np.float64)) + b_ih
    W_eff = W_eff.astype(np.float32)
    b_eff = b_eff.astype(np.float32)

    x = inputs.reshape(n_steps, E, N, F)
    h_state = state[-1].reshape(E, N).astype(np.float32)

    in_maps = []
    for k in range(N_CORES):
        es = [2 * k, 2 * k + 1]
        # x slice -> [W, e, c, p, f]
        xs = x[:, es].reshape(n_steps, E_LOC, NCHUNK, 128, F).astype(np.float32)
        xs = np.ascontiguousarray(xs)

        # weight stacks [128 (e,f), 6] f16
        we = np.zeros((128, 6), np.float16)
        wsign = np.array([1.0, -1.0, 1.0], np.float32)  # z-gate negated -> sigmoid gives zc
        we[0:64, 0:3] = (W_eff[es[0]] * wsign[:, None]).T.astype(np.float16)   # [f, g]
        we[64:128, 3:6] = (W_eff[es[1]] * wsign[:, None]).T.astype(np.float16)

        # per-partition scale/bias vectors [128, 8]
        erow = np.repeat(np.array(es), 64)  # 128 rows -> global e
        scb = np.zeros((128, 8), np.float32)
        scb[:, 0] = w_hh[erow, 0]
        scb[:, 1] = -w_hh[erow, 1]
        scb[:, 2] = w_hh[erow, 2]
        scb[:, 3] = b_eff[erow, 0] + b_hh[erow, 0]
        scb[:, 4] = -(b_eff[erow, 1] + b_hh[erow, 1])
        scb[:, 5] = b_hh[erow, 2]
        scb[:, 6] = b_eff[erow, 2]

        # h0 in chain layout [128, (q, c)]
        h0 = h_state[2 * k + _E_IDX, _N_IDX].reshape(128, 2 * NCHUNK).astype(np.float32)

        in_maps.append({"xin": xs, "we16": we, "scb": scb, "h0in": h0,
                        "ident": np.eye(128, dtype=np.float16)})
    return in_maps


def _unpack_outputs(results):
    """results: list of dicts with 'hout' [W, 128, 8] -> full (W, E, B, I, 1)."""
    out = np.zeros((W_STEPS, E, N), np.float32)
    for k in range(N_CORES):
        h = results[k]["hout"].reshape(W_STEPS, 128, 2, NCHUNK)
        out[:, 2 * k + _E_IDX, _N_IDX] = h
    return out.reshape(W_STEPS, E, B, I, 1)


def kernel(inputs, state, weight_linear, bias_linear, w_ih, w_hh, b_ih, b_hh):
    from concourse.bass_utils import run_bass_kernel_spmd

    nc = _get_program()
    in_maps = _host_prep(np.asarray(inputs, np.float32), np.asarray(state, np.float32),
                         np.asarray(weight_linear, np.float32), np.asarray(bias_linear, np.float32),
                         np.asarray(w_ih, np.float32), np.asarray(w_hh, np.float32),
                         np.asarray(b_ih, np.float32), np.asarray(b_hh, np.float32))
    res = run_bass_kernel_spmd(nc, in_maps, core_ids=list(range(N_CORES)))
    return _unpack_outputs(res.results)
